# revision 34
# baseline (speedup 1.0000x reference)
"""Causal self-attention kernel for 8 Trainium2 NeuronCores (fp8 redesign).

Problem: B=2, T=2048, C=1024, H=16 heads (HD=64).
  qkv = x @ w_attn + b_attn ; causal softmax attention ; y @ w_proj + b_proj

Sharding: tensor-parallel over heads. Core c owns heads {2c, 2c+1}. Each core
computes a partial projection outT_c = (y_local @ w_proj[rows_c])^T in f32,
DMA'd straight from PSUM; the host sums the 8 partials, adds the bias, and
transposes back.

Speed design (cost model: matmul = out_free_cols x cyc/col; bf16 1.0,
fp8e4m3+DoubleRow 0.5 with 2x128 contraction per instr):
  - QKV matmuls run fp8 DoubleRow (contraction 1024 = 4 pairs of 256): x and
    w_attn are host-folded [128, 2, .] fp8. DR can only write <=64 dst
    partitions at base 0, so Q/K land in a folded qTf/kTf [64, 2, BT] bf16
    layout - the fold lane is exactly the head split, which the S matmuls
    slice per head anyway. Q bias is applied during the PSUM->SBUF copy.
  - S = K^T Q stays bf16 (contraction 64 is DR-parity; out cols are the hard
    PE floor). The causal diagonal block is masked by a second PE matmul
    accumulating -30*lower_tri into the S PSUM (exp -> 0 in fp8), replacing
    all DVE mask work.
  - exp(S) writes fp8 P directly in a kb-pair-folded pT8 [128, 2h, 2t, cols]
    layout, so O' = P^T V runs fp8 DoubleRow with 256-key contraction (2x).
    exp chunks route mostly to ACT; a tunable fraction goes to DVE as a
    Schraudolph bit-trick exp (affine -> int32 -> bitcast f32 -> fp8, ~4%
    error, fine for softmax weights). The odd-kb tile's 128-col head start
    (fully masked) is zeroed once by a GPSIMD memset.
  - Denominators ride a second DR matmul (ones stationary) into a separate
    2KB PSUM region of the same [64, 1024] O' tile; normalize = DVE
    reciprocal + GPSIMD partition_broadcast + DVE multiply into yT bf16.
  - Projection stays bf16 (contraction 128 is DR-parity); its PSUM tiles DMA
    to DRAM as f32 directly (no PSUM->SBUF copy), host sums in f32.
  - V bias is folded into b_proj on the host (softmax weights sum to 1); the
    K bias is a softmax no-op and is dropped.
"""

import numpy as np

B, T, C, H = 2, 2048, 1024, 16
HD = C // H          # 64
NCORES = 8
HPC = H // NCORES    # 2 heads per core
BT = B * T           # 4096
NTCH = BT // 512     # 8 token chunks of 512
NPAIR = T // 256     # 8 kb-pairs per batch
NJC = T // 512       # 4 query chunks of 512 per batch

# pT8 packing: pair p occupies cols [POFF[p], POFF[p] + PU[p]) per (h, t)
PU = [2048 - 256 * p for p in range(NPAIR)]
POFF = [0] * NPAIR
for _p in range(1, NPAIR):
    POFF[_p] = POFF[_p - 1] + PU[_p - 1]
PTOT = POFF[-1] + PU[-1]     # 9216

QSCALE = 1.0 / 8.0   # 1/sqrt(HD), applied on-device to Q

# Schraudolph exp constants (probe-validated ~4-6% on fp8 output)
SCH_K1 = 12102203.1616
SCH_K2 = 1064866805.0

_CACHE = {}


def _build_program(exp_dve_every=0, proj_act_every=0, stop_after=None, rate0=0.5, rate1=0.5, qkv1_act=False, oq_first=False, pro_act=False, oburst=2, zip_pairs=3, zip_ratio=1, osb_bufs=4, x8_bufs=3):
    import collections

    import concourse.bacc as bacc
    import concourse.mybir as mybir
    import concourse.tile as tile
    from concourse.masks import make_upper_triangular

    f32 = mybir.dt.float32
    bf16 = mybir.dt.bfloat16
    fp8 = mybir.dt.float8e4
    i32 = mybir.dt.int32
    DR = mybir.MatmulPerfMode.DoubleRow
    Exp = mybir.ActivationFunctionType.Exp
    Mult = mybir.AluOpType.mult
    Ident = mybir.ActivationFunctionType.Identity
    Add = mybir.AluOpType.add

    nc = bacc.Bacc("TRN2", target_bir_lowering=False, debug=False,
                   num_devices=NCORES)

    # x folded for DR: x8[p, tch, j, t, tok] = xT[256j + 128t + p, 512tch+tok]
    x8_d = nc.dram_tensor("x8", [128, NTCH, 4, 2, 512], fp8,
                          kind="ExternalInput")
    # w folded: w8[p, j, t, col], col 0:128 q | 128:256 k | 256:384 v
    w8_d = nc.dram_tensor("w8", [128, 4, 2, 384], fp8, kind="ExternalInput")
    bq_d = nc.dram_tensor("bq", [64, 2], f32, kind="ExternalInput")
    xc_d = nc.dram_tensor("xc", [128, 8, B, 256], bf16, kind="ExternalInput")
    wc_d = nc.dram_tensor("wc", [128, 8, 384], bf16, kind="ExternalInput")
    wp_d = nc.dram_tensor("wp", [128, 8, 128], bf16, kind="ExternalInput")
    outT_d = nc.dram_tensor("outT", [C, BT], bf16, kind="ExternalOutput")

    with tile.TileContext(nc) as tc:
        with tc.tile_pool(name="const", bufs=1) as cst, \
             tc.tile_pool(name="pers", bufs=1) as pers, \
             tc.tile_pool(name="work", bufs=2) as work, \
             tc.tile_pool(name="ps", bufs=1, space="PSUM") as ps:

            # ---- critical-path loads ----
            w8 = cst.tile([128, 4, 2, 384], fp8, tag="w8")
            nc.sync.dma_start(w8[:], w8_d.ap())
            x8t = [None] * NTCH

            def load_x(tch):
                x8t[tch] = work.tile([128, 4, 2, 512], fp8, tag="x8",
                                     bufs=x8_bufs, name=f"x8{tch}")
                nc.scalar.dma_start(x8t[tch][:], x8_d.ap()[:, tch])

            load_x(0)
            bq = cst.tile([64, 2], f32, tag="bq")
            nc.sync.dma_start(bq[:], bq_d.ap())
            load_x(1)

            # ---- constants ----
            triU0f = cst.tile([128, 128], f32, tag="triU0f")
            make_upper_triangular(nc, triU0f[:], val=1.0, diag=False)
            tri = cst.tile([128, 128], bf16, tag="tri")
            nc.vector.tensor_copy(tri[:], triU0f[:])
            triU1f = cst.tile([128, 128], f32, tag="triU1f")
            make_upper_triangular(nc, triU1f[:], val=1.0, diag=True)
            negI = cst.tile([128, 128], bf16, tag="negI")
            identf = cst.tile([128, 128], f32, tag="identf")
            nc.vector.tensor_sub(identf[:], triU1f[:], triU0f[:])
            nc.vector.tensor_scalar_mul(negI[:], identf[:], -30.0)
            ones8 = cst.tile([128, 2, 16], fp8, tag="ones8")
            onesf = cst.tile([128, 2, 16], f32, tag="onesf")
            nc.vector.memset(onesf[:], 1.0)
            nc.vector.tensor_copy(ones8[:], onesf[:])
            # prewarm ACT exp table off the critical path
            warm = cst.tile([1, 2], f32, tag="warm")
            nc.scalar.activation(warm[:, 0:1], onesf[0:1, 0, 0:1], Exp)

            wp = cst.tile([128, 8, 128], bf16, tag="wp")
            xc = cst.tile([128, 8, B, 256], bf16, tag="xc")
            nc.sync.dma_start(xc[:], xc_d.ap())
            wc = cst.tile([128, 8, 384], bf16, tag="wc")
            nc.sync.dma_start(wc[:], wc_d.ap())
            vc_aug = cst.tile([128, B, 2, 2, 65], bf16, tag="vca")
            nc.vector.tensor_copy(
                vc_aug[:, :, :, :, 64:65],
                onesf[:, 0:1, 0:1].to_broadcast((128, B, 2, 2, 1)))
            pc0 = [None] * B
            pc1 = [None] * B

            # ---- persistent activations ----
            qTf = pers.tile([64, 2, BT], bf16, tag="qTf", name="qTf")
            kTf = pers.tile([64, 2, BT], bf16, tag="kTf", name="kTf")
            yT = pers.tile([128, BT], bf16, tag="yT", name="yT")
            v8 = [pers.tile([128, NPAIR, 2, 2, 64], fp8, tag=f"v8{b}",
                            name=f"v8{b}") for b in range(B)]
            pT8 = [pers.tile([128, 2, 2, PTOT], fp8, tag=f"pT8{b}",
                             name=f"pT8{b}") for b in range(B)]

            exp_ctr = [0]

            def emit_exp(sp, h2w, b, t, coff):
                # exp of sp[:, :, 0:w] (both heads) -> pT8[b][:, :, t, coff:]
                dst = pT8[b][:, :, t, coff:coff + h2w]
                src = sp[:, :, 0:h2w]
                exp_ctr[0] += 1
                if exp_dve_every and exp_ctr[0] % exp_dve_every == 0:
                    # Schraudolph split: DVE affine -> int32, Pool bitcast
                    sch = work.tile([128, 2, 512], i32, tag="sch", bufs=3,
                                    name=f"sch{exp_ctr[0]}")
                    nc.vector.tensor_scalar(sch[:, :, 0:h2w], src,
                                            SCH_K1, SCH_K2, Mult, Add)
                    nc.gpsimd.tensor_copy(dst, sch[:, :, 0:h2w].bitcast(f32))
                else:
                    nc.scalar.activation(dst, src, Exp)

            qkv_done = [-1]   # highest tch whose q/k/v copies are emitted

            def clean_qkv(b):
                # bf16 qkv for tokens [2048b, 2048b+256): overwrites the
                # folded qTf/kTf and fills vc_aug (early queries/keys get a
                # clean path; fp8 noise doesn't average out there)
                tc0 = 2048 * b
                for qk in range(2):
                    pqc = ps.tile([128, 256], f32, tag="sps", bufs=4,
                                  name=f"pqc{b}{qk}")
                    for cb in range(8):
                        nc.tensor.matmul(
                            pqc[:], wc[:, cb, 128 * qk:128 * qk + 128],
                            xc[:, cb, b, :],
                            start=(cb == 0), stop=(cb == 7))
                    dstf = qTf if qk == 0 else kTf
                    for dh in range(2):
                        if qk == 0:
                            nc.vector.tensor_scalar_add(
                                dstf[0:64, dh, tc0:tc0 + 256],
                                pqc[64 * dh:64 * dh + 64, :],
                                bq[:, dh:dh + 1])
                        else:
                            nc.vector.tensor_copy(
                                dstf[0:64, dh, tc0:tc0 + 256],
                                pqc[64 * dh:64 * dh + 64, :])
                    yield
                pvc = ps.tile([128, 2, 2, 64], f32, tag="sps", bufs=4,
                              name=f"pvc{b}")
                for t in range(2):
                    for cb in range(8):
                        nc.tensor.matmul(
                            pvc[:, t, :, :],
                            xc[:, cb, b, 128 * t:128 * t + 128],
                            wc[:, cb, 256:384],
                            start=(cb == 0), stop=(cb == 7))
                nc.vector.tensor_copy(vc_aug[:, b, :, :, 0:64], pvc[:])
                yield

            # ---------------- qkv ----------------
            def qkv_units(b, tchs=None, act_copies=False):
                for tch in (tchs if tchs is not None
                            else range(4 * b, 4 * b + 4)):
                    if x8t[tch] is None:
                        load_x(tch)
                    if tch + 1 < NTCH and x8t[tch + 1] is None:
                        load_x(tch + 1)
                    xt = x8t[tch]
                    tc0 = 512 * tch
                    # q then k: folded [64, 2, 512] psum layouts
                    pqk = [None, None]
                    for qk in range(2):
                        pqk[qk] = ps.tile([128, 2, 512], f32, tag="sps",
                                          bufs=4, name=f"pqk{qk}{tch}")
                        for dh in range(2):      # fold lane = head
                            for j in range(4):
                                nc.tensor.matmul(
                                    pqk[qk][0:64, dh, :],
                                    w8[:, j, :,
                                       128 * qk + 64 * dh:
                                       128 * qk + 64 * dh + 64],
                                    xt[:, j, :, :],
                                    start=(j == 0), stop=(j == 3),
                                    perf_mode=DR)
                        if qk == 0:
                            # Q bias per head lane during the copy
                            for dh in range(2):
                                if act_copies:
                                    nc.scalar.activation(
                                        qTf[0:64, dh, tc0:tc0 + 512],
                                        pqk[0][0:64, dh, :], Ident,
                                        bias=bq[:, dh:dh + 1],
                                        scale=float(QSCALE))
                                else:
                                    nc.vector.tensor_scalar(
                                        qTf[0:64, dh, tc0:tc0 + 512],
                                        pqk[0][0:64, dh, :], float(QSCALE),
                                        bq[:, dh:dh + 1], Mult, Add)
                        else:
                            if act_copies:
                                nc.scalar.copy(
                                    kTf[0:64, :, tc0:tc0 + 512],
                                    pqk[1][0:64, :, :])
                            else:
                                nc.vector.tensor_copy(
                                    kTf[0:64, :, tc0:tc0 + 512],
                                    pqk[1][0:64, :, :])
                        yield
                    # v: 8 blocks of 64 tokens, DR, out [64 tok, 128 vd]
                    pv = ps.tile([64, 2, 2, 2, 2, 64], f32, tag="sps", bufs=4,
                                 name=f"pv{tch}")
                    for bi in range(8):
                        phalf, plo, tt = bi % 2, bi // 4, (bi // 2) % 2
                        for j in range(4):
                            nc.tensor.matmul(
                                pv[:, phalf, plo, tt, :, :],
                                xt[:, j, :, 64 * bi:64 * bi + 64],
                                w8[:, j, :, 256:384],
                                start=(j == 0), stop=(j == 3), perf_mode=DR)
                    # copies: evens -> partitions 0:64, odds -> 64:128
                    pl = 2 * (tch % 4)
                    for half in range(2):
                        eng = nc.scalar.copy if act_copies else \
                            nc.vector.tensor_copy
                        eng(v8[b][64 * half:64 * half + 64,
                                  pl:pl + 2, :, :, :],
                            pv[:, half, :, :, :, :])
                    qkv_done[0] = max(qkv_done[0], tch)
                    yield

            # ---------------- attention ----------------
            def s_exp_units(b, pair):
                # zero the odd tile's masked 128-col head start
                nc.gpsimd.memset(
                    pT8[b][:, :, 1, POFF[pair]:POFF[pair] + 128], 0.0)
                base = 2048 * b
                for t in range(2):
                    kb = 2 * pair + t
                    k0 = base + 128 * kb
                    q0 = 128 * kb          # batch-relative first query
                    q = q0
                    first = True
                    while q < 2048:
                        w = min(512, 2048 - q)
                        need_tch((2048 * b + max(q + w, 128 * kb + 128) - 1)
                                 // 512)
                        sp = ps.tile([128, 2, 512], f32, tag="sps", bufs=4,
                                     name=f"sp{b}{pair}{t}{q}")
                        for h in range(2):
                            if first:
                                nc.tensor.matmul(
                                    sp[:, h, 0:128],
                                    kTf[0:64, h, k0:k0 + 128],
                                    qTf[0:64, h, base + q:base + q + 128],
                                    start=True, stop=False)
                                nc.tensor.matmul(
                                    sp[:, h, 0:128], tri[:], negI[:],
                                    start=False, stop=True)
                                if w > 128:
                                    nc.tensor.matmul(
                                        sp[:, h, 128:w],
                                        kTf[0:64, h, k0:k0 + 128],
                                        qTf[0:64, h,
                                            base + q + 128:base + q + w],
                                        start=True, stop=True)
                            else:
                                nc.tensor.matmul(
                                    sp[:, h, 0:w],
                                    kTf[0:64, h, k0:k0 + 128],
                                    qTf[0:64, h, base + q:base + q + w],
                                    start=True, stop=True)
                        emit_exp(sp, w, b, t, POFF[pair] + (q - 256 * pair))
                        if pair == 0 and first:
                            if t == 0:
                                pc0[b] = work.tile([128, 2, 256], bf16,
                                                   tag="pc0", bufs=2,
                                                   name=f"pc0{b}")
                                nc.scalar.activation(pc0[b][:],
                                                     sp[:, :, 0:256], Exp)
                            else:
                                pc1[b] = work.tile([128, 2, 128], bf16,
                                                   tag="pc1", bufs=2,
                                                   name=f"pc1{b}")
                                nc.scalar.activation(pc1[b][:],
                                                     sp[:, :, 0:128], Exp)
                        first = False
                        q += w
                        yield

            def clean_o(b):
                base = 2048 * b
                for h in range(2):
                    ovc = ps.tile([65, 256], f32, tag="sps", bufs=4,
                                  name=f"ovc{b}{h}")
                    nc.tensor.matmul(ovc[:, 0:256], vc_aug[:, b, 0, h, :],
                                     pc0[b][:, h, :],
                                     start=True, stop=False)
                    nc.tensor.matmul(ovc[:, 128:256], vc_aug[:, b, 1, h, :],
                                     pc1[b][:, h, :],
                                     start=False, stop=True)
                    d_sb = work.tile([1, 256], f32, tag="dsbc", bufs=2,
                                     name=f"dc{b}{h}")
                    nc.vector.reciprocal(d_sb[:], ovc[64:65, :])
                    rec = work.tile([64, 256], f32, tag="recc", bufs=2,
                                    name=f"recc{b}{h}")
                    nc.gpsimd.partition_broadcast(rec[:], d_sb[0:1, :])
                    nc.vector.tensor_mul(
                        yT[64 * h:64 * h + 64, base:base + 256],
                        ovc[0:64, :], rec[:])

            def o_units(b, jc):
                # O' for queries [512jc, 512jc+512), pairs 0..2jc+1, per head
                # norm emitted inline: a psum tile is never held across a
                # yield (pool slots rotate by allocation order)
                base = 2048 * b + 512 * jc
                for h in range(2):
                    ov = ps.tile([64, 1024], f32, tag="sps", bufs=4,
                                 name=f"ov{b}{jc}{h}")
                    for p in range(2 * jc + 2):
                        c0 = max(512 * jc - 256 * p, 0)
                        oc0 = max(256 * p - 512 * jc, 0)
                        width = 512 - oc0
                        rhs = pT8[b][:, h, :,
                                     POFF[p] + c0:POFF[p] + c0 + width]
                        st = (p == 0)
                        sp_ = (p == 2 * jc + 1)
                        nc.tensor.matmul(ov[0:64, oc0:oc0 + width],
                                         v8[b][:, p, :, h, :], rhs,
                                         start=st, stop=sp_, perf_mode=DR)
                        nc.tensor.matmul(ov[0:1, 512 + oc0:512 + oc0 + width],
                                         ones8[:, :, 0:1], rhs,
                                         start=st, stop=sp_, perf_mode=DR)
                    d_sb = work.tile([1, 512], f32, tag="dsb", bufs=4,
                                     name=f"d{b}{jc}{h}")
                    nc.vector.reciprocal(d_sb[:], ov[0:1, 512:1024])
                    rec = work.tile([64, 512], f32, tag="rec", bufs=4,
                                    name=f"rec{b}{jc}{h}")
                    nc.gpsimd.partition_broadcast(rec[:], d_sb[0:1, :])
                    lo = 256 if jc == 0 else 0
                    nc.vector.tensor_mul(
                        yT[64 * h:64 * h + 64, base + lo:base + 512],
                        ov[0:64, lo:512], rec[0:64, lo:512])
                    yield
                projq.append(proj_units(4 * b + jc))

            # ---------------- projection ----------------
            o_r = outT_d.ap().rearrange("(ob p) t -> p ob t", p=128)

            proj_ctr = [0]

            def proj_units(tch, tail=False):
                tc0 = 512 * tch
                for op in range(4):          # ot pairs
                    pp = ps.tile([128, 2, 512], f32, tag="sps", bufs=4,
                                 name=f"pp{tch}{op}")
                    osb = work.tile([128, 2, 512], bf16, tag="osb", bufs=osb_bufs,
                                    name=f"osb{tch}{op}")
                    for i in range(2):
                        nc.tensor.matmul(pp[:, i, :], wp[:, 2 * op + i, :],
                                         yT[:, tc0:tc0 + 512],
                                         start=True, stop=True)
                        proj_ctr[0] += 1
                        if tail and proj_ctr[0] % 2 == 0:
                            nc.scalar.copy(osb[:, i, :], pp[:, i, :])
                        else:
                            nc.vector.tensor_copy(osb[:, i, :], pp[:, i, :])
                    nc.sync.dma_start(
                        o_r[:, 2 * op:2 * op + 2, tc0:tc0 + 512], osb[:])
                    yield

            # ---------------- scheduling ----------------
            def drain_one(q):
                while q:
                    try:
                        next(q[0])
                        return True
                    except StopIteration:
                        q.popleft()
                return False

            def drain_q(q, n):
                for _ in range(n):
                    if not drain_one(q):
                        break

            fillerq = collections.deque()
            normq = collections.deque()
            projq = collections.deque()
            oq = collections.deque()

            def need_tch(n):
                # force-emit qkv units (in order) until tch n's outputs exist
                while qkv_done[0] < n:
                    if not drain_one(fillerq):
                        raise RuntimeError(f"need_tch({n}) starved")

            def drain_fillers(n):
                for _ in range(n):
                    q1, q2 = (oq, normq) if oq_first else (normq, oq)
                    if drain_one(q1):
                        continue
                    if drain_one(q2):
                        continue
                    if drain_one(fillerq):
                        continue
                    if not drain_one(projq):
                        break

            seg_ctr = [0]

            def attn_pairs(b, rate):
                for pair in range(NPAIR):
                    for _ in s_exp_units(b, pair):
                        seg_ctr[0] += 1
                        if rate >= 1:
                            drain_fillers(rate)
                        elif seg_ctr[0] % int(round(1 / rate)) == 0:
                            drain_fillers(1)
                    if pair == 0:
                        clean_o(b)
                    if pair % 2 == 1:
                        need_tch(4 * b + pair // 2)
                        oq.append(o_units(b, pair // 2))
                        drain_fillers(oburst)
                    yield pair

            def attn_batch(b, rate):
                for _ in attn_pairs(b, rate):
                    pass

            # qkv(0) fully, then attention(0) with qkv(1) as filler,
            # then attention(1) with proj(0) as filler, then tails
            for _ in qkv_units(0, tchs=(0, 1), act_copies=True):
                pass
            for _ in clean_qkv(0):
                pass
            nc.sync.dma_start(wp[:], wp_d.ap())
            fillerq.append(qkv_units(0, tchs=(2, 3), act_copies=pro_act))
            fillerq.append(qkv_units(1, act_copies=qkv1_act))
            fillerq.append(clean_qkv(1))
            if stop_after != "qkv0":
                g0 = attn_pairs(0, rate0)
                for pair in g0:
                    if pair == NPAIR - 1 - zip_pairs:
                        break
                # all of qkv(1) + clean(1) must be emitted before any
                # batch-1 S matmul (clean overwrites qTf/kTf cols)
                drain_q(fillerq, 10 ** 6)
                if stop_after != "attn0":
                    g1 = attn_pairs(1, rate1)
                    for _ in g0:
                        for _ in range(zip_ratio):
                            next(g1, None)
                    for _ in g1:
                        pass
                    drain_q(oq, 10 ** 6)
                    drain_q(normq, 10 ** 6)
                    drain_q(projq, 10 ** 6)

    nc.compile()
    return nc


def _prep_inputs(x, w_attn, b_attn, w_proj):
    import ml_dtypes
    F8 = ml_dtypes.float8_e4m3
    BF = ml_dtypes.bfloat16
    xT = x.reshape(BT, C).T                       # [C, BT] f32
    # x8[p, tch, j, t, tok] = xT[256j + 128t + p, 512tch + tok]
    x8 = np.ascontiguousarray(
        xT.reshape(4, 2, 128, NTCH, 512).transpose(2, 3, 0, 1, 4).astype(F8))
    scale = np.float32(1.0 / np.sqrt(HD))
    # xc[p, cb, b, tok] = xT[128cb + p, 2048b + tok], bf16 clean slice
    xc = np.ascontiguousarray(
        xT.reshape(8, 128, B, T)[:, :, :, 0:256]
        .transpose(1, 0, 2, 3).astype(BF))
    in_maps = []
    for c in range(NCORES):
        lo = 128 * c
        wq = w_attn[:, lo:lo + 128]
        wk = w_attn[:, C + lo:C + lo + 128]
        wv = w_attn[:, 2 * C + lo:2 * C + lo + 128]
        wsel = np.concatenate([wq, wk, wv], axis=1)       # [C, 384]
        w8 = np.ascontiguousarray(
            wsel.reshape(4, 2, 128, 384).transpose(2, 0, 1, 3).astype(F8))
        # bq[p, dh] = b_attn[lo + 64*dh + p] * scale
        bq = np.ascontiguousarray(
            (b_attn[lo:lo + 128] * scale).reshape(2, 64).T
            .astype(np.float32))
        wp = np.ascontiguousarray(
            w_proj[lo:lo + 128, :].reshape(128, 8, 128).astype(BF))
        wsel_c = np.concatenate([wq * scale, wk, wv], axis=1)
        wc = np.ascontiguousarray(
            wsel_c.reshape(8, 128, 384).transpose(1, 0, 2).astype(BF))
        in_maps.append({"x8": x8, "w8": w8, "bq": bq, "wp": wp,
                        "xc": xc, "wc": wc})
    return in_maps


def kernel(x, w_attn, b_attn, w_proj, b_proj, _trace=False):
    from concourse.bass_utils import run_bass_kernel_spmd

    x = np.asarray(x, dtype=np.float32)
    w_attn = np.asarray(w_attn, dtype=np.float32)
    b_attn = np.asarray(b_attn, dtype=np.float32)
    w_proj = np.asarray(w_proj, dtype=np.float32)
    b_proj = np.asarray(b_proj, dtype=np.float32)

    if "nc" not in _CACHE:
        _CACHE["nc"] = _build_program()
    nc = _CACHE["nc"]

    in_maps = _prep_inputs(x, w_attn, b_attn, w_proj)
    res = run_bass_kernel_spmd(nc, in_maps, core_ids=list(range(NCORES)),
                               trace=_trace)
    _CACHE["last_results"] = res

    outT = res.results[0]["outT"].astype(np.float64)
    for c in range(1, NCORES):
        outT += res.results[c]["outT"]
    # V bias folded on host: y = y_attn + bv exactly (softmax weights sum
    # to 1), so out += bv @ w_proj lands in the bias term
    b_eff = b_proj + b_attn[2 * C:3 * C].astype(np.float64) @ \
        w_proj.astype(np.float64)
    out = outT.T.astype(np.float32) + b_eff[None, :].astype(np.float32)
    return out.reshape(B, T, C)
